# revision 9
# baseline (speedup 1.0000x reference)
"""Trainium2 Bass kernel for single-head self-attention over image tokens.

Reference computation (per batch element b of 4):
    xf   = x[b] viewed as [N=4096 tokens, C=256]          (x stored [C, H*W] = xf.T)
    qkv  = xf @ w_qkv.T                                   -> q, k, v each [N, 512]
    sim  = (q * 64**-0.5) @ k.T                           [N, N]
    attn = softmax(sim, axis=-1)
    out  = (attn @ v) @ w_out.T + b_out + xf              [N, C]

Sharding: 8 cores = 4 batches x 2 query-row halves (2048 rows each). Each core
computes k/v for its full batch but q/out only for its half. No collectives.
Each core's x is host-rotated so its query half is always columns 0:2048
(softmax over keys is permutation invariant, so key order doesn't matter).

All heavy matmuls run in bf16 (both PE operands must share a dtype class on
TRN2; bf16 streams 1 col/cycle with fast FWL weight loads). x and w_qkv are
pre-rounded to bf16 on the host; on-chip intermediates (qT/kT/v/pT) get
rounded by the PSUM->SBUF copy or activation that produces them. The final
projection + softmax normalization stay f32r, and the residual is added from
a full-f32 copy of x, so the end-to-end relative error stays ~2.3e-3.

v2 layout: everything resident in SBUF (bf16 halves the footprint):
    x [C, 4096] -> qT [512, 2048], kT [512, 4096], v-tiles [128, 512] x 32.
    Slice-major attention: for each 512-query slice, outT [d, i] accumulates
    over ALL 32 key chunks in 4 PSUM banks (no SBUF accumulator), simT/exp
    feed it one chunk late so the PE never waits on the activation, and the
    softmax denominator accumulates on GpSimd into an f32r running sum,
    reduced by a single ones-matmul per slice. Per-slice finalize (reciprocal
    via rank-1 broadcast matmul + Newton on DVE, f32r projection, residual,
    DMA out) is deferred into the next slice's PE stream. Dummy matmuls at
    t=0 warm the PE clock (HAM un-throttles after ~3.4us busy) while the
    first DMA pieces land; x/w stream in 512-column pieces across the
    sync/gpsimd/scalar queues so real work starts ~4us in.
"""

import hashlib
import os
import shutil

import numpy as np

import concourse.bacc as bacc
import concourse.tile as tile
import concourse.mybir as mybir
from concourse.bass_utils import run_bass_kernel_spmd


def _install_neff_cache():
    """Disk-cache walrus NEFF compiles keyed on the BIR content hash.

    The axon PJRT path recompiles the NEFF in every fresh process (~minutes);
    the build here is deterministic, so identical BIR -> identical NEFF.
    """
    try:
        import concourse.bass2jax as bass2jax
        orig = bass2jax.compile_bir_kernel
        if getattr(orig, "_neff_cache_wrapped", False):
            return
        cache_dir = os.path.expanduser("~/.neuron-compile-cache/bass-neff")

        def cached(bir_json, tmpdir, neff_name="file.neff"):
            try:
                key = hashlib.sha256(
                    bir_json if isinstance(bir_json, bytes)
                    else bir_json.encode()).hexdigest()
                hit = os.path.join(cache_dir, key + ".neff")
                dst = os.path.join(tmpdir, neff_name)
                if os.path.exists(hit):
                    shutil.copyfile(hit, dst)
                    return dst
                neff = orig(bir_json, tmpdir, neff_name=neff_name)
                os.makedirs(cache_dir, exist_ok=True)
                tmp = hit + ".tmp%d" % os.getpid()
                shutil.copyfile(neff, tmp)
                os.replace(tmp, hit)
                return neff
            except Exception:
                return orig(bir_json, tmpdir, neff_name=neff_name)

        cached._neff_cache_wrapped = True
        bass2jax.compile_bir_kernel = cached
    except Exception:
        pass


_install_neff_cache()

F32 = mybir.dt.float32
F32R = mybir.dt.float32r
BF16 = mybir.dt.bfloat16
Exp = mybir.ActivationFunctionType.Exp

B = 4
C = 256          # model dim (2 chunks of 128)
N = 4096         # tokens per batch (64*64)
HALF = N // 2    # query rows per core
INNER = 512      # qkv inner dim (4 chunks of 128)
SCALE = 0.125    # 64 ** -0.5

NCORES = 8
NSL = 4          # i slices per core
SW = HALF // NSL # 512 query columns per slice
NJC = N // 128   # 32 key chunks of 128
KLATE = 3        # po drain runs this many chunks behind sim/exp


def build_nc():
    nc = bacc.Bacc(None)
    x_b = nc.declare_dram_parameter("x_b", [C, N], BF16, isOutput=False)
    xq_f = nc.declare_dram_parameter("xq_f", [C, HALF], F32, isOutput=False)
    wqkvT = nc.declare_dram_parameter("wqkvT", [C, 3 * INNER], BF16, isOutput=False)
    woutT = nc.declare_dram_parameter("woutT", [INNER, C], F32R, isOutput=False)
    bout = nc.declare_dram_parameter("bout", [2, 128, 1], F32, isOutput=False)
    out = nc.declare_dram_parameter("out", [C, HALF], F32, isOutput=True)

    mm = nc.tensor.matmul

    with tile.TileContext(nc) as tc:
        with tc.tile_pool(name="const", bufs=1) as const, \
             tc.tile_pool(name="work", bufs=2) as work, \
             tc.tile_pool(name="pp", bufs=1, space="PSUM") as pp:

            def psum(tag, shape=(128, SW), bufs=3, name=None):
                return pp.tile(list(shape), F32, tag=tag, bufs=bufs,
                               name=name or tag)

            # ---- PE warmup: dummy matmuls while the first DMAs land ----
            warm = const.tile([128, SW], BF16, tag="warm", name="warm")
            nc.vector.memset(warm, 0.0)
            for w in range(8):
                ps = psum("sim", name="ps_warm")
                mm(ps, warm[:, :128], warm, start=True, stop=True)

            # ---- input DMAs, split across queues for parallel startup ----
            # scalar queue: weights (q part first, kv part behind it)
            wq = [const.tile([128, 3 * INNER], BF16, tag=f"wq{cc}",
                             name=f"wq{cc}") for cc in range(2)]
            for cc in range(2):
                nc.scalar.dma_start(wq[cc][:, :INNER],
                                    wqkvT[cc * 128:(cc + 1) * 128, :INNER])
            for cc in range(2):
                nc.scalar.dma_start(wq[cc][:, INNER:],
                                    wqkvT[cc * 128:(cc + 1) * 128, INNER:])
            # sync + gpsimd queues: x in 512-column pieces (one queue per cc)
            xt = [const.tile([128, N], BF16, tag=f"xt{cc}", name=f"xt{cc}")
                  for cc in range(2)]
            for blk in range(8):
                sl = slice(blk * 512, (blk + 1) * 512)
                nc.sync.dma_start(xt[0][:, sl], x_b[0:128, sl])
                nc.gpsimd.dma_start(xt[1][:, sl], x_b[128:256, sl])

            qT = [const.tile([128, HALF], BF16, tag=f"qt{d}", name=f"qt{d}")
                  for d in range(4)]
            kt = [const.tile([128, N], BF16, tag=f"kt{d}", name=f"kt{d}")
                  for d in range(4)]
            vt = [const.tile([128, INNER], BF16, tag=f"vt{nj}", name=f"vt{nj}")
                  for nj in range(NJC)]

            ones_col = const.tile([128, 1], F32R, tag="ones_col", name="ones_col")
            ones_row = const.tile([1, 128], F32R, tag="ones_row", name="ones_row")
            ones_f = const.tile([128, 1], F32, tag="ones_f", name="ones_f")
            ones_rf = const.tile([1, 128], F32, tag="ones_rf", name="ones_rf")
            nc.vector.memset(ones_f, 1.0)
            nc.vector.tensor_copy(ones_col, ones_f)
            nc.vector.memset(ones_rf, 1.0)
            nc.vector.tensor_copy(ones_row, ones_rf)

            # ---- qT production (x cols 0:2048, q rows of wq) ----
            for nb in range(HALF // 512):
                for d in range(4):
                    ps = psum("sim", name="ps_q")
                    for cc in range(2):
                        mm(ps, wq[cc][:, d * 128:(d + 1) * 128],
                           xt[cc][:, nb * 512:(nb + 1) * 512],
                           start=(cc == 0), stop=(cc == 1))
                    nc.scalar.copy(qT[d][:, nb * 512:(nb + 1) * 512], ps)

            # ---- kT / v production over all 8 x blocks ----
            for blk in range(8):
                bsl = slice(blk * 512, (blk + 1) * 512)
                for d in range(4):
                    ps = psum("sim", name="ps_k")
                    for cc in range(2):
                        mm(ps, wq[cc][:, INNER + d * 128:INNER + (d + 1) * 128],
                           xt[cc][:, bsl],
                           start=(cc == 0), stop=(cc == 1))
                    nc.scalar.copy(kt[d][:, bsl], ps)
                for sub in range(4):
                    nj = blk * 4 + sub
                    ps = psum("sim", shape=(128, INNER), name="ps_v")
                    for cc in range(2):
                        mm(ps, xt[cc][:, nj * 128:(nj + 1) * 128],
                           wq[cc][:, 2 * INNER:3 * INNER],
                           start=(cc == 0), stop=(cc == 1))
                    nc.scalar.copy(vt[nj], ps)

            # ---- late DMAs (residual x in f32, w_out, bias) on vector ----
            wo = [const.tile([128, C], F32R, tag=f"wo{d}", name=f"wo{d}")
                  for d in range(4)]
            for d in range(4):
                nc.scalar.dma_start(wo[d], woutT[d * 128:(d + 1) * 128, :])
            xqt = [const.tile([128, HALF], F32, tag=f"xq{cc}", name=f"xq{cc}")
                   for cc in range(2)]
            bt = [const.tile([128, 1], F32, tag=f"b{cc}", name=f"b{cc}")
                  for cc in range(2)]
            for cc in range(2):
                nc.gpsimd.dma_start(xqt[cc], xq_f[cc * 128:(cc + 1) * 128, :])
                nc.scalar.dma_start(bt[cc], bout[cc])
            for cc in range(2):
                nc.vector.tensor_scalar_add(xqt[cc], xqt[cc], bt[cc])

            # ---- attention, slice-major; finalize deferred one slice ----
            deferred = {}

            def finalize(s, po):
                """Emit slice s finalize in parts; called during slice s+1
                (or inline for the last slice). Parts are keyed by the j
                position of the next slice where each lands on the PE."""
                sl = slice(s * SW, (s + 1) * SW)
                l_rs = work.tile([1, SW], F32R, tag="l_rs", bufs=2, name="l_rs")
                otr = [work.tile([128, SW], F32R, tag=f"otr{d}", bufs=1,
                                 name=f"otr{d}") for d in range(4)]
                bc = work.tile([128, SW], F32, tag="bc", bufs=2, name="bc")
                rsc = work.tile([128, SW], F32, tag="rsc", bufs=2, name="rsc")

                def part0(pl=deferred["pl"]):
                    nc.scalar.copy(l_rs, pl)
                    for d in range(4):
                        nc.vector.tensor_copy(otr[d], po[d])
                    pb = psum("sim", name="pb")
                    mm(pb, ones_row, l_rs, start=True, stop=True)
                    nc.vector.reciprocal_approx_accurate(bc, pb, rsc)

                def part_cc(cc):
                    pf = psum("sim", name="pf")
                    for d in range(4):
                        mm(pf, wo[d][:, cc * 128:(cc + 1) * 128], otr[d],
                           start=(d == 0), stop=(d == 3))
                    fo = work.tile([128, SW], F32, tag="fo", bufs=2, name="fo")
                    nc.vector.tensor_mul(fo, pf, bc)
                    nc.vector.tensor_add(fo, fo, xqt[cc][:, sl])
                    nc.sync.dma_start(out[cc * 128:(cc + 1) * 128, sl], fo)

                return [part0, lambda: part_cc(0), lambda: part_cc(1)]

            for s in range(NSL):
                sl = slice(s * SW, (s + 1) * SW)
                po = [psum(f"po{d}", bufs=1, name=f"po{d}") for d in range(4)]
                lacc = work.tile([128, SW], F32R, tag="lacc", bufs=2,
                                 name="lacc")
                pts = []
                parts = deferred.pop("parts", None)

                def drain(j):
                    pt = pts[j]
                    for d in range(4):
                        mm(po[d], vt[j][:, d * 128:(d + 1) * 128], pt,
                           start=(j == 0), stop=(j == NJC - 1))

                for j in range(NJC):
                    ps = psum("sim", name="ps_s")
                    for d in range(4):
                        mm(ps, kt[d][:, j * 128:(j + 1) * 128], qT[d][:, sl],
                           start=(d == 0), stop=(d == 3))
                    pt = work.tile([128, SW], BF16, tag="pt", bufs=6, name="pt")
                    nc.scalar.activation(pt, ps, Exp, scale=SCALE)
                    pts.append(pt)
                    if j == 0:
                        nc.gpsimd.tensor_copy(lacc, pt)
                    else:
                        nc.gpsimd.tensor_add(lacc, lacc, pt)
                    if j >= KLATE:
                        drain(j - KLATE)
                    # previous slice's finalize, spread into this stream
                    if parts and j in (2, 4, 6):
                        parts.pop(0)()
                for j in range(NJC - KLATE, NJC):
                    drain(j)

                # denominator for this slice: one ones-matmul on the f32r sum
                pl = psum("pl", shape=(1, SW), bufs=1, name="pl")
                mm(pl, ones_col, lacc, start=True, stop=True)
                deferred["pl"] = pl
                parts_s = finalize(s, po)
                if s == NSL - 1:
                    for p in parts_s:
                        p()
                else:
                    deferred["parts"] = parts_s

    nc.finalize()
    return nc


_NC_CACHE = None


def _get_nc():
    global _NC_CACHE
    if _NC_CACHE is None:
        _NC_CACHE = build_nc()
    return _NC_CACHE


def _round_f32r(a):
    """fp32 -> float32r rounding (round-half-even on the low 12 mantissa
    bits), matching the hardware's fp32_to_fp32r conversion."""
    bits = np.ascontiguousarray(a, dtype=np.float32).view(np.uint32)
    rem = bits & np.uint32(0xFFF)
    base = bits & np.uint32(0xFFFFF000)
    up = (rem > 0x800) | ((rem == 0x800) & (((bits >> np.uint32(12)) & np.uint32(1)) == 1))
    return (base + np.where(up, np.uint32(0x1000), np.uint32(0))).view(np.float32)


def prepare_in_maps(x, w_qkv, w_out, b_out):
    import ml_dtypes
    bf16 = ml_dtypes.bfloat16
    x = np.asarray(x, dtype=np.float32)
    w_qkv = np.asarray(w_qkv, dtype=np.float32)
    w_out = np.asarray(w_out, dtype=np.float32)
    b_out = np.asarray(b_out, dtype=np.float32)

    xr = x.reshape(B, C, N)
    wqkvT = np.ascontiguousarray(w_qkv.T).astype(bf16)   # [C, 1536]
    woutT = _round_f32r(np.ascontiguousarray(w_out.T))   # [512, C]
    bout = np.ascontiguousarray(b_out.reshape(2, 128, 1))

    in_maps = []
    for c in range(NCORES):
        b, h = divmod(c, 2)
        if h == 0:
            x_rot = xr[b]
        else:  # rotate so this core's query half sits in columns 0:HALF
            x_rot = np.concatenate([xr[b][:, HALF:], xr[b][:, :HALF]], axis=1)
        in_maps.append({
            "x_b": x_rot.astype(bf16),
            "xq_f": np.ascontiguousarray(x_rot[:, :HALF]),
            "wqkvT": wqkvT,
            "woutT": woutT,
            "bout": bout,
        })
    return in_maps


def postprocess(results):
    outs = [results[c]["out"] for c in range(NCORES)]
    full = np.stack([np.concatenate([outs[2 * b], outs[2 * b + 1]], axis=1)
                     for b in range(B)])               # [B, C, N]
    return full.reshape(B, C, 64, 64).astype(np.float32)


def kernel(x, w_qkv, w_out, b_out):
    in_maps = prepare_in_maps(x, w_qkv, w_out, b_out)
    res = run_bass_kernel_spmd(_get_nc(), in_maps, core_ids=list(range(NCORES)))
    return postprocess(res.results)


# revision 10
# speedup vs baseline: 1.0987x; 1.0987x over previous
"""Trainium2 Bass kernel for single-head self-attention over image tokens.

Reference computation (per batch element b of 4):
    xf   = x[b] viewed as [N=4096 tokens, C=256]          (x stored [C, H*W] = xf.T)
    qkv  = xf @ w_qkv.T                                   -> q, k, v each [N, 512]
    sim  = (q * 64**-0.5) @ k.T                           [N, N]
    attn = softmax(sim, axis=-1)
    out  = (attn @ v) @ w_out.T + b_out + xf              [N, C]

Sharding: 8 cores = 4 batches x 2 query-row halves (2048 rows each). Each core
computes k/v for its full batch but q/out only for its half. No collectives.
Each core's x is host-rotated so its query half is always columns 0:2048
(softmax over keys is permutation invariant, so key order doesn't matter).

Matmul operands use float32r: fp32 with the mantissa rounded to 11 bits
(round-half-even on the low 12 bits, same bit layout as fp32), which streams
1 PE column/cycle instead of 4 for plain fp32. x and the weights are
pre-rounded on the host and DMAed straight into float32r tiles; on-chip
intermediates (qT/kT/v/pT) get rounded by the PSUM->SBUF copy or activation
that produces them.

On-chip layout keeps everything in the "transposed activation" orientation so
no PE transposes are needed:
    qT [512, 2048] and kT [512, N] come straight out of the QKV projection
    (x's HBM layout [C, N] is already the rhs/lhsT the PE wants);
    v [N, 512] comes from the same projection with x slices as the stationary
    operand. simT [j, i] = kT.T @ qT, pT = exp(0.125*simT), then
    outT [d, i] += v_j.T @ pT accumulates in PSUM per 1024-column j-superblock
    and the softmax denominator l[1, i] += ones.T @ (pT pairs summed on
    GpSimd). Normalization is folded in at the end of the last superblock,
    per query slice: recip(l) via a fast Newton iteration on the DVE after a
    K=1 rank-1 broadcast matmul, multiplied into the final projection output.
"""

import hashlib
import os
import shutil

import numpy as np

import concourse.bacc as bacc
import concourse.tile as tile
import concourse.mybir as mybir
from concourse.bass_utils import run_bass_kernel_spmd


def _install_neff_cache():
    """Disk-cache walrus NEFF compiles keyed on the BIR content hash.

    The axon PJRT path recompiles the NEFF in every fresh process (~minutes);
    the build here is deterministic, so identical BIR -> identical NEFF.
    """
    try:
        import concourse.bass2jax as bass2jax
        orig = bass2jax.compile_bir_kernel
        if getattr(orig, "_neff_cache_wrapped", False):
            return
        cache_dir = os.path.expanduser("~/.neuron-compile-cache/bass-neff")

        def cached(bir_json, tmpdir, neff_name="file.neff"):
            try:
                key = hashlib.sha256(
                    bir_json if isinstance(bir_json, bytes)
                    else bir_json.encode()).hexdigest()
                hit = os.path.join(cache_dir, key + ".neff")
                dst = os.path.join(tmpdir, neff_name)
                if os.path.exists(hit):
                    shutil.copyfile(hit, dst)
                    return dst
                neff = orig(bir_json, tmpdir, neff_name=neff_name)
                os.makedirs(cache_dir, exist_ok=True)
                tmp = hit + ".tmp%d" % os.getpid()
                shutil.copyfile(neff, tmp)
                os.replace(tmp, hit)
                return neff
            except Exception:
                return orig(bir_json, tmpdir, neff_name=neff_name)

        cached._neff_cache_wrapped = True
        bass2jax.compile_bir_kernel = cached
    except Exception:
        pass


_install_neff_cache()

F32 = mybir.dt.float32
F32R = mybir.dt.float32r
BF16 = mybir.dt.bfloat16
Exp = mybir.ActivationFunctionType.Exp

B = 4
C = 256          # model dim (2 chunks of 128)
N = 4096         # tokens per batch (64*64)
HALF = N // 2    # query rows per core
INNER = 512      # qkv inner dim (4 chunks of 128)
SCALE = 0.125    # 64 ** -0.5

NCORES = 8
NJB = 4          # j superblocks per batch
JBW = N // NJB   # 1024 key columns per superblock
NSL = 4          # i slices per core
SW = HALF // NSL # 512 query columns per slice


def build_nc(n=N, njb=NJB, nsl=NSL):
    half = n // 2
    jbw = n // njb
    assert half % SW == 0 and jbw % SW == 0 and jbw % 256 == 0
    nc = bacc.Bacc(None)
    x_r = nc.declare_dram_parameter("x_r", [C, n], BF16, isOutput=False)
    xq_f = nc.declare_dram_parameter("xq_f", [C, half], F32, isOutput=False)
    wqkvT = nc.declare_dram_parameter("wqkvT", [C, 3 * INNER], BF16, isOutput=False)
    woutT = nc.declare_dram_parameter("woutT", [INNER, C], F32R, isOutput=False)
    bout = nc.declare_dram_parameter("bout", [2, 128, 1], F32, isOutput=False)
    out = nc.declare_dram_parameter("out", [C, half], F32, isOutput=True)

    mm = nc.tensor.matmul

    with tile.TileContext(nc) as tc:
        with tc.tile_pool(name="const", bufs=1) as const, \
             tc.tile_pool(name="stream", bufs=1) as stream, \
             tc.tile_pool(name="work", bufs=2) as work, \
             tc.tile_pool(name="pp", bufs=1, space="PSUM") as pp:

            # ---- resident weights: direct f32r DMA (host pre-rounded) ----
            wq = []
            for cc in range(2):
                t = const.tile([128, 3 * INNER], BF16, tag=f"wq{cc}", name=f"wq{cc}")
                nc.sync.dma_start(t, wqkvT[cc * 128:(cc + 1) * 128, :])
                wq.append(t)

            def xchunk(cc, col, width):
                """x chunk [128, width] in f32r, shares slots with xjb tiles."""
                t = stream.tile([128, width], BF16, tag=f"xjb{cc}", bufs=2,
                                name=f"xjb{cc}", padded_shape=[128, jbw])
                nc.sync.dma_start(t, x_r[cc * 128:(cc + 1) * 128, col:col + width])
                return t

            qT = [const.tile([128, half], BF16, tag=f"qt{d}", name=f"qt{d}")
                  for d in range(4)]
            ot = [const.tile([128, half], F32, tag=f"ot{d}", name=f"ot{d}")
                  for d in range(4)]
            l_sb = const.tile([1, half], F32, tag="l_sb", name="l_sb")

            ones_col_f = const.tile([128, 1], F32, tag="ones_col_f", name="ones_col_f")
            nc.vector.memset(ones_col_f, 1.0)
            ones_col = const.tile([128, 1], BF16, tag="ones_col", name="ones_col")
            nc.vector.tensor_copy(ones_col, ones_col_f)
            ones_row_f = const.tile([1, 128], F32, tag="ones_row_f", name="ones_row_f")
            nc.vector.memset(ones_row_f, 1.0)
            ones_row = const.tile([1, 128], F32R, tag="ones_row", name="ones_row")
            nc.vector.tensor_copy(ones_row, ones_row_f)

            # ---- qT production from x columns 0:half ----
            wo = []
            xqt = []
            bt = []
            qcw = min(1024, half)
            for qch in range(half // qcw):
                xch = [xchunk(cc, qch * qcw, qcw) for cc in range(2)]
                for d in range(4):
                    for nb in range(qcw // SW):
                        ns = qch * (qcw // SW) + nb
                        ps = pp.tile([128, SW], F32, tag="sim", bufs=3, name="ps_q")
                        for cc in range(2):
                            mm(ps, wq[cc][:, d * 128:(d + 1) * 128],
                               xch[cc][:, nb * SW:(nb + 1) * SW],
                               start=(cc == 0), stop=(cc == 1))
                        nc.scalar.copy(qT[d][:, ns * SW:(ns + 1) * SW], ps)
            # final-phase constants, off the startup critical path
            # (vector-queue DMAs so the sync queue stays free for x chunks)
            for d in range(4):
                t = const.tile([128, C], F32R, tag=f"wo{d}", name=f"wo{d}")
                nc.scalar.dma_start(t, woutT[d * 128:(d + 1) * 128, :])
                wo.append(t)
            for cc in range(2):
                t = const.tile([128, half], F32, tag=f"xq{cc}", name=f"xq{cc}")
                nc.scalar.dma_start(t, xq_f[cc * 128:(cc + 1) * 128, :])
                xqt.append(t)
            for cc in range(2):
                t = const.tile([128, 1], F32, tag=f"b{cc}", name=f"b{cc}")
                nc.scalar.dma_start(t, bout[cc])
                bt.append(t)

            # residual-with-bias: xqt <- xqt + b
            for cc in range(2):
                nc.vector.tensor_scalar_add(xqt[cc], xqt[cc], bt[cc])

            # ---- attention over j superblocks ----
            deferred = []  # denominator work deferred into later PE streams
            for jb in range(njb):
                xjb = [xchunk(cc, jb * jbw, jbw) for cc in range(2)]
                # kT for this superblock: [512, jbw]
                kt = [stream.tile([128, jbw], BF16, tag=f"kt{d}", bufs=1,
                                  name=f"kt{d}") for d in range(4)]
                for d in range(4):
                    for nb in range(jbw // SW):
                        ps = pp.tile([128, SW], F32, tag="sim", bufs=3, name="ps_k")
                        for cc in range(2):
                            mm(ps, wq[cc][:, INNER + d * 128:INNER + (d + 1) * 128],
                               xjb[cc][:, nb * SW:(nb + 1) * SW],
                               start=(cc == 0), stop=(cc == 1))
                        nc.scalar.copy(kt[d][:, nb * SW:(nb + 1) * SW], ps)
                # v for this superblock: [jbw, 512] (token rows on partitions)
                vt = []
                for nj in range(jbw // 128):
                    t = stream.tile([128, INNER], BF16, tag=f"vt{nj}", bufs=1,
                                    name=f"vt{nj}")
                    ps = pp.tile([128, INNER], F32, tag="sim", bufs=3, name="ps_v")
                    for cc in range(2):
                        mm(ps, xjb[cc][:, nj * 128:(nj + 1) * 128],
                           wq[cc][:, 2 * INNER:3 * INNER],
                           start=(cc == 0), stop=(cc == 1))
                    nc.scalar.copy(t, ps)
                    vt.append(t)

                for fn in deferred:
                    fn()
                deferred.clear()

                nj8 = jbw // 128
                for s in range(nsl):
                    sl = slice(s * SW, (s + 1) * SW)
                    po = [pp.tile([128, SW], F32, tag=f"po{d}", bufs=1,
                                  name=f"po{d}") for d in range(4)]
                    pl = pp.tile([1, SW], F32, tag="aux", bufs=1, name="pl")
                    pts = []

                    sums = []  # binary tree of pT partial sums (DVE)

                    def tree_add(t):
                        sums.append([t, 0])
                        while len(sums) >= 2 and sums[-1][1] == sums[-2][1]:
                            a, lv = sums.pop()
                            b, _ = sums.pop()
                            t2 = work.tile([128, SW], BF16, tag="pt2", bufs=4,
                                           name="pt2")
                            nc.vector.tensor_add(t2, b, a)
                            sums.append([t2, lv + 1])

                    last_jb = jb == njb - 1

                    def l_update(jb=jb, sl=sl, pl=pl):
                        if jb == 0:
                            nc.vector.tensor_copy(l_sb[:, sl], pl)
                        else:
                            nc.vector.tensor_add(l_sb[:, sl], l_sb[:, sl], pl)

                    def drain_j8(j8):
                        # outT + denominator work for chunk j8 (emitted one
                        # chunk late so the PE never waits on the exp)
                        pt = pts[j8]
                        for d in range(4):
                            mm(po[d], vt[j8][:, d * 128:(d + 1) * 128], pt,
                               start=(j8 == 0), stop=(j8 == nj8 - 1))
                        if last_jb:
                            # inline pT pair sums: the finalize below needs l
                            # with no deferral room
                            if j8 % 2 == 1:
                                pt2 = work.tile([128, SW], BF16, tag="pt2",
                                                bufs=4, name="pt2")
                                nc.vector.tensor_add(pt2, pts[j8 - 1], pt)
                                mm(pl, ones_col, pt2,
                                   start=(j8 == 1), stop=(j8 == nj8 - 1))
                        else:
                            # tree-sum the pT chunks on the DVE; the single
                            # ones-matmul + l update are deferred into a later
                            # PE stream so the PE never waits on the adder tree
                            tree_add(pt)
                            if j8 == nj8 - 1:
                                assert len(sums) == 1
                                pt8 = sums[0][0]

                                def flush(pl=pl, pt8=pt8, upd=l_update):
                                    mm(pl, ones_col, pt8, start=True, stop=True)
                                    upd()
                                deferred.append(flush)

                    for j8 in range(nj8):
                        ps = pp.tile([128, SW], F32, tag="sim", bufs=3, name="ps_s")
                        for d in range(4):
                            mm(ps, kt[d][:, j8 * 128:(j8 + 1) * 128], qT[d][:, sl],
                               start=(d == 0), stop=(d == 3))
                        pt = work.tile([128, SW], BF16, tag="pt", bufs=4, name="pt")
                        nc.scalar.activation(pt, ps, Exp, scale=SCALE)
                        pts.append(pt)
                        if j8 > 0:
                            drain_j8(j8 - 1)
                        if j8 == 2:
                            for fn in deferred:
                                fn()
                            deferred.clear()
                    drain_j8(nj8 - 1)
                    if last_jb:
                        l_update()
                    for d in range(4):
                        if jb == 0:
                            nc.vector.tensor_copy(ot[d][:, sl], po[d])
                        else:
                            nc.vector.tensor_add(ot[d][:, sl], ot[d][:, sl], po[d])

                    if jb == njb - 1:
                        # ---- finalize slice s: normalize + project + out ----
                        l_rs = work.tile([1, SW], F32R, tag="l_rs", bufs=2,
                                         name="l_rs")
                        nc.scalar.copy(l_rs, l_sb[:, sl])
                        pb = pp.tile([128, SW], F32, tag="sim", bufs=3, name="pb")
                        mm(pb, ones_row, l_rs, start=True, stop=True)
                        bc = work.tile([128, SW], F32, tag="bc", bufs=2, name="bc")
                        rsc = work.tile([128, SW], F32, tag="rsc", bufs=2,
                                        name="rsc")
                        nc.vector.reciprocal_approx_accurate(bc, pb, rsc)
                        otr = [work.tile([128, SW], F32R, tag=f"otr{d}", bufs=1,
                                         name=f"otr{d}") for d in range(4)]
                        for d in range(4):
                            nc.scalar.copy(otr[d], ot[d][:, sl])
                        for cc in range(2):
                            pf = pp.tile([128, SW], F32, tag="sim", bufs=3,
                                         name="pf")
                            for d in range(4):
                                mm(pf, wo[d][:, cc * 128:(cc + 1) * 128], otr[d],
                                   start=(d == 0), stop=(d == 3))
                            fo = work.tile([128, SW], F32, tag="fo", bufs=2,
                                           name="fo")
                            nc.vector.tensor_mul(fo, pf, bc)
                            nc.vector.tensor_add(fo, fo, xqt[cc][:, sl])
                            nc.sync.dma_start(out[cc * 128:(cc + 1) * 128, sl], fo)

    nc.finalize()
    return nc


_NC_CACHE = None


def _get_nc():
    global _NC_CACHE
    if _NC_CACHE is None:
        _NC_CACHE = build_nc()
    return _NC_CACHE


def _round_f32r(a):
    """fp32 -> float32r rounding (round-half-even on the low 12 mantissa
    bits), matching the hardware's fp32_to_fp32r conversion."""
    bits = np.ascontiguousarray(a, dtype=np.float32).view(np.uint32)
    rem = bits & np.uint32(0xFFF)
    base = bits & np.uint32(0xFFFFF000)
    up = (rem > 0x800) | ((rem == 0x800) & (((bits >> np.uint32(12)) & np.uint32(1)) == 1))
    return (base + np.where(up, np.uint32(0x1000), np.uint32(0))).view(np.float32)


def prepare_in_maps(x, w_qkv, w_out, b_out):
    import ml_dtypes
    bf16 = ml_dtypes.bfloat16
    x = np.asarray(x, dtype=np.float32)
    w_qkv = np.asarray(w_qkv, dtype=np.float32)
    w_out = np.asarray(w_out, dtype=np.float32)
    b_out = np.asarray(b_out, dtype=np.float32)

    xr = x.reshape(B, C, N)
    wqkvT = np.ascontiguousarray(w_qkv.T).astype(bf16)   # [C, 1536]
    woutT = _round_f32r(np.ascontiguousarray(w_out.T))   # [512, C]
    bout = np.ascontiguousarray(b_out.reshape(2, 128, 1))

    in_maps = []
    for c in range(NCORES):
        b, h = divmod(c, 2)
        if h == 0:
            x_rot = xr[b]
        else:  # rotate so this core's query half sits in columns 0:HALF
            x_rot = np.concatenate([xr[b][:, HALF:], xr[b][:, :HALF]], axis=1)
        in_maps.append({
            "x_r": x_rot.astype(bf16),
            "xq_f": np.ascontiguousarray(x_rot[:, :HALF]),
            "wqkvT": wqkvT,
            "woutT": woutT,
            "bout": bout,
        })
    return in_maps


def postprocess(results):
    outs = [results[c]["out"] for c in range(NCORES)]
    full = np.stack([np.concatenate([outs[2 * b], outs[2 * b + 1]], axis=1)
                     for b in range(B)])               # [B, C, N]
    return full.reshape(B, C, 64, 64).astype(np.float32)


def kernel(x, w_qkv, w_out, b_out):
    in_maps = prepare_in_maps(x, w_qkv, w_out, b_out)
    res = run_bass_kernel_spmd(_get_nc(), in_maps, core_ids=list(range(NCORES)))
    return postprocess(res.results)



# revision 14
# speedup vs baseline: 1.1663x; 1.0615x over previous
"""Trainium2 Bass kernel for single-head self-attention over image tokens.

Reference computation (per batch element b of 4):
    xf   = x[b] viewed as [N=4096 tokens, C=256]          (x stored [C, H*W] = xf.T)
    qkv  = xf @ w_qkv.T                                   -> q, k, v each [N, 512]
    sim  = (q * 64**-0.5) @ k.T                           [N, N]
    attn = softmax(sim, axis=-1)
    out  = (attn @ v) @ w_out.T + b_out + xf              [N, C]

Sharding: 8 cores = 4 batches x 2 query-row halves (2048 rows each). Each core
computes k/v for its full batch but q/out only for its half. No collectives.
Each core's x is host-rotated so its query half is always columns 0:2048
(softmax over keys is permutation invariant, so key order doesn't matter).

All heavy matmuls run in bf16 (both PE operands must share a dtype class on
TRN2; bf16 streams 1 col/cycle with fast FWL weight loads). x and w_qkv are
pre-rounded to bf16 on the host; on-chip intermediates (qT/kT/v/pT) get
rounded by the PSUM->SBUF copy or activation that produces them. The final
projection + softmax normalization stay f32r, and the residual is added from
a full-f32 copy of x, so the end-to-end relative error stays ~2.3e-3.

v2 layout: everything resident in SBUF (bf16 halves the footprint):
    x [C, 4096] -> qT [512, 2048], kT [512, 4096], v-tiles [128, 512] x 32.
    Slice-major attention: for each 512-query slice, outT [d, i] accumulates
    over ALL 32 key chunks in 4 PSUM banks (no SBUF accumulator), simT/exp
    feed it one chunk late so the PE never waits on the activation, and the
    softmax denominator accumulates on GpSimd into an f32r running sum,
    reduced by a single ones-matmul per slice. Per-slice finalize (reciprocal
    via rank-1 broadcast matmul + Newton on DVE, f32r projection, residual,
    DMA out) is deferred into the next slice's PE stream. Dummy matmuls at
    t=0 warm the PE clock (HAM un-throttles after ~3.4us busy) while the
    first DMA pieces land; x/w stream in 512-column pieces across the
    sync/gpsimd/scalar queues so real work starts ~4us in.
"""

import hashlib
import os
import shutil

import numpy as np

import concourse.bacc as bacc
import concourse.tile as tile
import concourse.mybir as mybir
from concourse.bass_utils import run_bass_kernel_spmd


def _install_neff_cache():
    """Disk-cache walrus NEFF compiles keyed on the BIR content hash.

    The axon PJRT path recompiles the NEFF in every fresh process (~minutes);
    the build here is deterministic, so identical BIR -> identical NEFF.
    """
    try:
        import concourse.bass2jax as bass2jax
        orig = bass2jax.compile_bir_kernel
        if getattr(orig, "_neff_cache_wrapped", False):
            return
        cache_dir = os.path.expanduser("~/.neuron-compile-cache/bass-neff")

        def cached(bir_json, tmpdir, neff_name="file.neff"):
            try:
                key = hashlib.sha256(
                    bir_json if isinstance(bir_json, bytes)
                    else bir_json.encode()).hexdigest()
                hit = os.path.join(cache_dir, key + ".neff")
                dst = os.path.join(tmpdir, neff_name)
                if os.path.exists(hit):
                    shutil.copyfile(hit, dst)
                    return dst
                neff = orig(bir_json, tmpdir, neff_name=neff_name)
                os.makedirs(cache_dir, exist_ok=True)
                tmp = hit + ".tmp%d" % os.getpid()
                shutil.copyfile(neff, tmp)
                os.replace(tmp, hit)
                return neff
            except Exception:
                return orig(bir_json, tmpdir, neff_name=neff_name)

        cached._neff_cache_wrapped = True
        bass2jax.compile_bir_kernel = cached
    except Exception:
        pass


_install_neff_cache()

F32 = mybir.dt.float32
F32R = mybir.dt.float32r
BF16 = mybir.dt.bfloat16
Exp = mybir.ActivationFunctionType.Exp

B = 4
C = 256          # model dim (2 chunks of 128)
N = 4096         # tokens per batch (64*64)
HALF = N // 2    # query rows per core
INNER = 512      # qkv inner dim (4 chunks of 128)
SCALE = 0.125    # 64 ** -0.5

NCORES = 8
NSL = 4          # i slices per core
SW = HALF // NSL # 512 query columns per slice
NJC = N // 128   # 32 key chunks of 128
KLATE = 3        # po drain runs this many chunks behind sim/exp


def build_nc():
    nc = bacc.Bacc(None)
    x_b = nc.declare_dram_parameter("x_b", [C, N], BF16, isOutput=False)
    xq_f = nc.declare_dram_parameter("xq_f", [C, HALF], F32, isOutput=False)
    wqkvT = nc.declare_dram_parameter("wqkvT", [C, 3 * INNER], BF16, isOutput=False)
    woutT = nc.declare_dram_parameter("woutT", [INNER, C], F32R, isOutput=False)
    bout = nc.declare_dram_parameter("bout", [2, 128, 1], F32, isOutput=False)
    out = nc.declare_dram_parameter("out", [C, HALF], F32, isOutput=True)

    mm = nc.tensor.matmul

    with tile.TileContext(nc) as tc:
        with tc.tile_pool(name="const", bufs=1) as const, \
             tc.tile_pool(name="work", bufs=2) as work, \
             tc.tile_pool(name="pp", bufs=1, space="PSUM") as pp:

            def psum(tag, shape=(128, SW), bufs=3, name=None):
                return pp.tile(list(shape), F32, tag=tag, bufs=bufs,
                               name=name or tag)

            # ---- PE warmup: dummy matmuls while the first DMAs land ----
            warm = const.tile([128, SW], BF16, tag="warm", name="warm")
            nc.vector.memset(warm, 0.0)
            for w in range(8):
                ps = psum("sim", name="ps_warm")
                mm(ps, warm[:, :128], warm, start=True, stop=True)

            # ---- input DMAs, split across queues for parallel startup ----
            # scalar queue: weights (q part first, kv part behind it), then
            # the cc=1 half of x; sync queue: the cc=0 half of x
            wq = [const.tile([128, 3 * INNER], BF16, tag=f"wq{cc}",
                             name=f"wq{cc}") for cc in range(2)]
            for cc in range(2):
                nc.scalar.dma_start(wq[cc][:, :INNER],
                                    wqkvT[cc * 128:(cc + 1) * 128, :INNER])
            for cc in range(2):
                nc.scalar.dma_start(wq[cc][:, INNER:],
                                    wqkvT[cc * 128:(cc + 1) * 128, INNER:])
            xt = [const.tile([128, N], BF16, tag=f"xt{cc}", name=f"xt{cc}")
                  for cc in range(2)]
            for blk in range(8):
                sl = slice(blk * 512, (blk + 1) * 512)
                nc.sync.dma_start(xt[0][:, sl], x_b[0:128, sl])
                nc.scalar.dma_start(xt[1][:, sl], x_b[128:256, sl])

            qT = [const.tile([128, HALF], BF16, tag=f"qt{d}", name=f"qt{d}")
                  for d in range(4)]
            kt = [const.tile([128, N], BF16, tag=f"kt{d}", name=f"kt{d}")
                  for d in range(4)]
            vt = [const.tile([128, INNER], BF16, tag=f"vt{nj}", name=f"vt{nj}")
                  for nj in range(NJC)]

            ones_col = const.tile([128, 1], F32R, tag="ones_col", name="ones_col")
            ones_row = const.tile([1, 128], F32R, tag="ones_row", name="ones_row")
            ones_f = const.tile([128, 1], F32, tag="ones_f", name="ones_f")
            ones_rf = const.tile([1, 128], F32, tag="ones_rf", name="ones_rf")
            nc.vector.memset(ones_f, 1.0)
            nc.vector.tensor_copy(ones_col, ones_f)
            nc.vector.memset(ones_rf, 1.0)
            nc.vector.tensor_copy(ones_row, ones_rf)

            # ---- qT production (x cols 0:2048, q rows of wq) ----
            for nb in range(HALF // 512):
                for d in range(4):
                    ps = psum("sim", name="ps_q")
                    for cc in range(2):
                        mm(ps, wq[cc][:, d * 128:(d + 1) * 128],
                           xt[cc][:, nb * 512:(nb + 1) * 512],
                           start=(cc == 0), stop=(cc == 1))
                    nc.scalar.copy(qT[d][:, nb * 512:(nb + 1) * 512], ps)

            # ---- kT / v production over all 8 x blocks ----
            for blk in range(8):
                bsl = slice(blk * 512, (blk + 1) * 512)
                for d in range(4):
                    ps = psum("sim", name="ps_k")
                    for cc in range(2):
                        mm(ps, wq[cc][:, INNER + d * 128:INNER + (d + 1) * 128],
                           xt[cc][:, bsl],
                           start=(cc == 0), stop=(cc == 1))
                    nc.scalar.copy(kt[d][:, bsl], ps)
                for sub in range(4):
                    nj = blk * 4 + sub
                    ps = psum("sim", shape=(128, INNER), name="ps_v")
                    for cc in range(2):
                        mm(ps, xt[cc][:, nj * 128:(nj + 1) * 128],
                           wq[cc][:, 2 * INNER:3 * INNER],
                           start=(cc == 0), stop=(cc == 1))
                    nc.scalar.copy(vt[nj], ps)

            # ---- late DMAs (residual x in f32, w_out, bias) on vector ----
            wo = [const.tile([128, C], F32R, tag=f"wo{d}", name=f"wo{d}")
                  for d in range(4)]
            for d in range(4):
                nc.scalar.dma_start(wo[d], woutT[d * 128:(d + 1) * 128, :])
            xqt = [const.tile([128, HALF], F32, tag=f"xq{cc}", name=f"xq{cc}")
                   for cc in range(2)]
            bt = [const.tile([128, 1], F32, tag=f"b{cc}", name=f"b{cc}")
                  for cc in range(2)]
            for cc in range(2):
                nc.sync.dma_start(xqt[cc], xq_f[cc * 128:(cc + 1) * 128, :])
                nc.scalar.dma_start(bt[cc], bout[cc])
            for cc in range(2):
                nc.vector.tensor_scalar_add(xqt[cc], xqt[cc], bt[cc])

            # ---- attention, slice-major; finalize deferred one slice ----
            deferred = {}

            def finalize(s, po):
                """Emit slice s finalize in parts; called during slice s+1
                (or inline for the last slice). Parts are keyed by the j
                position of the next slice where each lands on the PE."""
                sl = slice(s * SW, (s + 1) * SW)
                l_rs = work.tile([1, SW], F32R, tag="l_rs", bufs=2, name="l_rs")
                otr = [work.tile([128, SW], F32R, tag=f"otr{d}", bufs=1,
                                 name=f"otr{d}") for d in range(4)]
                bc = work.tile([128, SW], F32, tag="bc", bufs=2, name="bc")
                rsc = work.tile([128, SW], F32, tag="rsc", bufs=2, name="rsc")

                def part0(pl=deferred["pl"]):
                    nc.scalar.copy(l_rs, pl)
                    for d in range(4):
                        nc.vector.tensor_copy(otr[d], po[d])
                    pb = psum("sim", name="pb")
                    mm(pb, ones_row, l_rs, start=True, stop=True)
                    nc.vector.reciprocal_approx_accurate(bc, pb, rsc)

                def part_cc(cc):
                    pf = psum("sim", name="pf")
                    for d in range(4):
                        mm(pf, wo[d][:, cc * 128:(cc + 1) * 128], otr[d],
                           start=(d == 0), stop=(d == 3))
                    fo = work.tile([128, SW], F32, tag="fo", bufs=2, name="fo")
                    nc.vector.tensor_mul(fo, pf, bc)
                    nc.vector.tensor_add(fo, fo, xqt[cc][:, sl])
                    nc.sync.dma_start(out[cc * 128:(cc + 1) * 128, sl], fo)

                return [part0, lambda: part_cc(0), lambda: part_cc(1)]

            for s in range(NSL):
                sl = slice(s * SW, (s + 1) * SW)
                po = [psum(f"po{d}", bufs=1, name=f"po{d}") for d in range(4)]
                lacc = work.tile([128, SW], F32R, tag="lacc", bufs=2,
                                 name="lacc")
                pts = []
                parts = deferred.pop("parts", None)

                def drain(j):
                    pt = pts[j]
                    for d in range(4):
                        mm(po[d], vt[j][:, d * 128:(d + 1) * 128], pt,
                           start=(j == 0), stop=(j == NJC - 1))

                for j in range(NJC):
                    ps = psum("sim", name="ps_s")
                    for d in range(4):
                        mm(ps, kt[d][:, j * 128:(j + 1) * 128], qT[d][:, sl],
                           start=(d == 0), stop=(d == 3))
                    pt = work.tile([128, SW], BF16, tag="pt", bufs=6, name="pt")
                    nc.scalar.activation(pt, ps, Exp, scale=SCALE)
                    pts.append(pt)
                    if j == 0:
                        nc.vector.tensor_copy(lacc, pt)
                    else:
                        nc.vector.tensor_add(lacc, lacc, pt)
                    if j >= KLATE:
                        drain(j - KLATE)
                    # previous slice's finalize, spread into this stream
                    if parts and j in (2, 4, 6):
                        parts.pop(0)()
                for j in range(NJC - KLATE, NJC):
                    drain(j)

                # denominator for this slice: one ones-matmul on the f32r sum
                pl = psum("pl", shape=(1, SW), bufs=1, name="pl")
                mm(pl, ones_col, lacc, start=True, stop=True)
                deferred["pl"] = pl
                parts_s = finalize(s, po)
                if s == NSL - 1:
                    for p in parts_s:
                        p()
                else:
                    deferred["parts"] = parts_s

    nc.finalize()
    return nc


_NC_CACHE = None


def _get_nc():
    global _NC_CACHE
    if _NC_CACHE is None:
        _NC_CACHE = build_nc()
    return _NC_CACHE


def _round_f32r(a):
    """fp32 -> float32r rounding (round-half-even on the low 12 mantissa
    bits), matching the hardware's fp32_to_fp32r conversion."""
    bits = np.ascontiguousarray(a, dtype=np.float32).view(np.uint32)
    rem = bits & np.uint32(0xFFF)
    base = bits & np.uint32(0xFFFFF000)
    up = (rem > 0x800) | ((rem == 0x800) & (((bits >> np.uint32(12)) & np.uint32(1)) == 1))
    return (base + np.where(up, np.uint32(0x1000), np.uint32(0))).view(np.float32)


def prepare_in_maps(x, w_qkv, w_out, b_out):
    import ml_dtypes
    bf16 = ml_dtypes.bfloat16
    x = np.asarray(x, dtype=np.float32)
    w_qkv = np.asarray(w_qkv, dtype=np.float32)
    w_out = np.asarray(w_out, dtype=np.float32)
    b_out = np.asarray(b_out, dtype=np.float32)

    xr = x.reshape(B, C, N)
    wqkvT = np.ascontiguousarray(w_qkv.T).astype(bf16)   # [C, 1536]
    woutT = _round_f32r(np.ascontiguousarray(w_out.T))   # [512, C]
    bout = np.ascontiguousarray(b_out.reshape(2, 128, 1))

    in_maps = []
    for c in range(NCORES):
        b, h = divmod(c, 2)
        if h == 0:
            x_rot = xr[b]
        else:  # rotate so this core's query half sits in columns 0:HALF
            x_rot = np.concatenate([xr[b][:, HALF:], xr[b][:, :HALF]], axis=1)
        in_maps.append({
            "x_b": x_rot.astype(bf16),
            "xq_f": np.ascontiguousarray(x_rot[:, :HALF]),
            "wqkvT": wqkvT,
            "woutT": woutT,
            "bout": bout,
        })
    return in_maps


def postprocess(results):
    outs = [results[c]["out"] for c in range(NCORES)]
    full = np.stack([np.concatenate([outs[2 * b], outs[2 * b + 1]], axis=1)
                     for b in range(B)])               # [B, C, N]
    return full.reshape(B, C, 64, 64).astype(np.float32)


def kernel(x, w_qkv, w_out, b_out):
    in_maps = prepare_in_maps(x, w_qkv, w_out, b_out)
    res = run_bass_kernel_spmd(_get_nc(), in_maps, core_ids=list(range(NCORES)))
    return postprocess(res.results)


# revision 18
# speedup vs baseline: 1.1910x; 1.0212x over previous
"""Trainium2 Bass kernel for single-head self-attention over image tokens.

Reference computation (per batch element b of 4):
    xf   = x[b] viewed as [N=4096 tokens, C=256]          (x stored [C, H*W] = xf.T)
    qkv  = xf @ w_qkv.T                                   -> q, k, v each [N, 512]
    sim  = (q * 64**-0.5) @ k.T                           [N, N]
    attn = softmax(sim, axis=-1)
    out  = (attn @ v) @ w_out.T + b_out + xf              [N, C]

Sharding: 8 cores = 4 batches x 2 query-row halves (2048 rows each). Each core
computes k/v for its full batch but q/out only for its half. No collectives.
Each core's x is host-rotated so its query half is always columns 0:2048
(softmax over keys is permutation invariant, so key order doesn't matter).

All heavy matmuls run in bf16 (both PE operands must share a dtype class on
TRN2; bf16 streams 1 col/cycle with fast FWL weight loads). x and w_qkv are
pre-rounded to bf16 on the host; on-chip intermediates (qT/kT/v/pT) get
rounded by the PSUM->SBUF copy or activation that produces them. The final
projection + softmax normalization stay f32r, and the residual is added from
a full-f32 copy of x, so the end-to-end relative error stays ~2.3e-3.

v2 layout: everything resident in SBUF (bf16 halves the footprint):
    x [C, 4096] -> qT [512, 2048], kT [512, 4096], v-tiles [128, 512] x 32.
    Slice-major attention: for each 512-query slice, outT [d, i] accumulates
    over ALL 32 key chunks in 4 PSUM banks (no SBUF accumulator), simT/exp
    feed it one chunk late so the PE never waits on the activation, and the
    softmax denominator accumulates on GpSimd into an f32r running sum,
    reduced by a single ones-matmul per slice. Per-slice finalize (reciprocal
    via rank-1 broadcast matmul + Newton on DVE, f32r projection, residual,
    DMA out) is deferred into the next slice's PE stream. Dummy matmuls at
    t=0 warm the PE clock (HAM un-throttles after ~3.4us busy) while the
    first DMA pieces land; x/w stream in 512-column pieces across the
    sync/gpsimd/scalar queues so real work starts ~4us in.
"""

import hashlib
import os
import shutil

import numpy as np

import concourse.bacc as bacc
import concourse.tile as tile
import concourse.mybir as mybir
from concourse.bass_utils import run_bass_kernel_spmd


def _install_neff_cache():
    """Disk-cache walrus NEFF compiles keyed on the BIR content hash.

    The axon PJRT path recompiles the NEFF in every fresh process (~minutes);
    the build here is deterministic, so identical BIR -> identical NEFF.
    """
    try:
        import concourse.bass2jax as bass2jax
        orig = bass2jax.compile_bir_kernel
        if getattr(orig, "_neff_cache_wrapped", False):
            return
        cache_dir = os.path.expanduser("~/.neuron-compile-cache/bass-neff")

        def cached(bir_json, tmpdir, neff_name="file.neff"):
            try:
                key = hashlib.sha256(
                    bir_json if isinstance(bir_json, bytes)
                    else bir_json.encode()).hexdigest()
                hit = os.path.join(cache_dir, key + ".neff")
                dst = os.path.join(tmpdir, neff_name)
                if os.path.exists(hit):
                    shutil.copyfile(hit, dst)
                    return dst
                neff = orig(bir_json, tmpdir, neff_name=neff_name)
                os.makedirs(cache_dir, exist_ok=True)
                tmp = hit + ".tmp%d" % os.getpid()
                shutil.copyfile(neff, tmp)
                os.replace(tmp, hit)
                return neff
            except Exception:
                return orig(bir_json, tmpdir, neff_name=neff_name)

        cached._neff_cache_wrapped = True
        bass2jax.compile_bir_kernel = cached
    except Exception:
        pass


_install_neff_cache()

F32 = mybir.dt.float32
F32R = mybir.dt.float32r
BF16 = mybir.dt.bfloat16
Exp = mybir.ActivationFunctionType.Exp

B = 4
C = 256          # model dim (2 chunks of 128)
N = 4096         # tokens per batch (64*64)
HALF = N // 2    # query rows per core
INNER = 512      # qkv inner dim (4 chunks of 128)
SCALE = 0.125    # 64 ** -0.5

NCORES = 8
NSL = 4          # i slices per core
SW = HALF // NSL # 512 query columns per slice
NJC = N // 128   # 32 key chunks of 128
KLATE = 3        # po drain runs this many chunks behind sim/exp


def build_nc():
    nc = bacc.Bacc(None)
    x_b = nc.declare_dram_parameter("x_b", [C, N], BF16, isOutput=False)
    xq_f = nc.declare_dram_parameter("xq_f", [C, HALF], F32, isOutput=False)
    wqkvT = nc.declare_dram_parameter("wqkvT", [C, 3 * INNER], BF16, isOutput=False)
    woutT = nc.declare_dram_parameter("woutT", [INNER, C], F32R, isOutput=False)
    bout = nc.declare_dram_parameter("bout", [2, 128, 1], F32, isOutput=False)
    out = nc.declare_dram_parameter("out", [C, HALF], F32, isOutput=True)

    mm = nc.tensor.matmul

    with tile.TileContext(nc) as tc:
        with tc.tile_pool(name="const", bufs=1) as const, \
             tc.tile_pool(name="work", bufs=2) as work, \
             tc.tile_pool(name="pp", bufs=1, space="PSUM") as pp:

            def psum(tag, shape=(128, SW), bufs=3, name=None):
                return pp.tile(list(shape), F32, tag=tag, bufs=bufs,
                               name=name or tag)

            # ---- PE warmup: dummy matmuls while the first DMAs land ----
            # (the DGE rings take ~9us to deliver the first input bytes; the
            # HAM clock-gate needs ~3.4us of sustained PE busy to un-throttle,
            # and re-throttles after a ~3.4us idle window, so keep the PE
            # spinning until real work can start)
            warm = const.tile([128, SW], BF16, tag="warm", name="warm")
            nc.vector.memset(warm, 0.0)
            for w in range(24):
                ps = psum("sim", name="ps_warm")
                mm(ps, warm[:, :128], warm, start=True, stop=True)

            # ---- input DMAs, split across queues for parallel startup ----
            # scalar queue: q weights, then cc=1 x for the query half, then
            # kv weights, then the rest; sync queue: all of cc=0 x
            wq = [const.tile([128, 3 * INNER], BF16, tag=f"wq{cc}",
                             name=f"wq{cc}") for cc in range(2)]
            xt = [const.tile([128, N], BF16, tag=f"xt{cc}", name=f"xt{cc}")
                  for cc in range(2)]

            def xdma(cc, blk):
                sl = slice(blk * 512, (blk + 1) * 512)
                q = nc.sync if cc == 0 else nc.scalar
                q.dma_start(xt[cc][:, sl], x_b[cc * 128:(cc + 1) * 128, sl])

            for cc in range(2):
                nc.scalar.dma_start(wq[cc][:, :INNER],
                                    wqkvT[cc * 128:(cc + 1) * 128, :INNER])
            for blk in range(8):
                xdma(0, blk)
            for blk in range(4):
                xdma(1, blk)
            for cc in range(2):
                nc.scalar.dma_start(wq[cc][:, INNER:],
                                    wqkvT[cc * 128:(cc + 1) * 128, INNER:])
            for blk in range(4, 8):
                xdma(1, blk)

            qT = [const.tile([128, HALF], BF16, tag=f"qt{d}", name=f"qt{d}")
                  for d in range(4)]
            kt = [const.tile([128, N], BF16, tag=f"kt{d}", name=f"kt{d}")
                  for d in range(4)]
            vt = [const.tile([128, INNER], BF16, tag=f"vt{nj}", name=f"vt{nj}")
                  for nj in range(NJC)]

            ones_col = const.tile([128, 1], F32R, tag="ones_col", name="ones_col")
            ones_row = const.tile([1, 128], F32R, tag="ones_row", name="ones_row")
            ones_f = const.tile([128, 1], F32, tag="ones_f", name="ones_f")
            ones_rf = const.tile([1, 128], F32, tag="ones_rf", name="ones_rf")
            nc.vector.memset(ones_f, 1.0)
            nc.vector.tensor_copy(ones_col, ones_f)
            nc.vector.memset(ones_rf, 1.0)
            nc.vector.tensor_copy(ones_row, ones_rf)

            def sbcopy(dst, src, d):
                """PSUM->SBUF copies alternate scalar/vector so neither
                engine limits the production phases."""
                if d % 2 == 0:
                    nc.scalar.copy(dst, src)
                else:
                    nc.vector.tensor_copy(dst, src)

            # ---- qT production (x cols 0:2048, q rows of wq) ----
            for nb in range(HALF // 512):
                for d in range(4):
                    ps = psum("sim", name="ps_q")
                    for cc in range(2):
                        mm(ps, wq[cc][:, d * 128:(d + 1) * 128],
                           xt[cc][:, nb * 512:(nb + 1) * 512],
                           start=(cc == 0), stop=(cc == 1))
                    sbcopy(qT[d][:, nb * 512:(nb + 1) * 512], ps, d)

            # ---- kT / v production over all 8 x blocks ----
            for blk in range(8):
                bsl = slice(blk * 512, (blk + 1) * 512)
                for d in range(4):
                    ps = psum("sim", name="ps_k")
                    for cc in range(2):
                        mm(ps, wq[cc][:, INNER + d * 128:INNER + (d + 1) * 128],
                           xt[cc][:, bsl],
                           start=(cc == 0), stop=(cc == 1))
                    nc.scalar.copy(kt[d][:, bsl], ps)
                for sub in range(4):
                    nj = blk * 4 + sub
                    ps = psum("sim", shape=(128, INNER), name="ps_v")
                    for cc in range(2):
                        mm(ps, xt[cc][:, nj * 128:(nj + 1) * 128],
                           wq[cc][:, 2 * INNER:3 * INNER],
                           start=(cc == 0), stop=(cc == 1))
                    nc.vector.tensor_copy(vt[nj], ps)

            # ---- late DMAs (residual x in f32, w_out, bias) on vector ----
            wo = [const.tile([128, C], F32R, tag=f"wo{d}", name=f"wo{d}")
                  for d in range(4)]
            for d in range(4):
                nc.scalar.dma_start(wo[d], woutT[d * 128:(d + 1) * 128, :])
            xqt = [const.tile([128, HALF], F32, tag=f"xq{cc}", name=f"xq{cc}")
                   for cc in range(2)]
            bt = [const.tile([128, 1], F32, tag=f"b{cc}", name=f"b{cc}")
                  for cc in range(2)]
            for cc in range(2):
                nc.sync.dma_start(xqt[cc], xq_f[cc * 128:(cc + 1) * 128, :])
                nc.scalar.dma_start(bt[cc], bout[cc])
            for cc in range(2):
                nc.vector.tensor_scalar_add(xqt[cc], xqt[cc], bt[cc])

            # ---- attention, slice-major; finalize deferred one slice ----
            deferred = {}

            def finalize(s, po):
                """Emit slice s finalize in parts; called during slice s+1
                (or inline for the last slice). Parts are keyed by the j
                position of the next slice where each lands on the PE."""
                sl = slice(s * SW, (s + 1) * SW)
                l_rs = work.tile([1, SW], F32R, tag="l_rs", bufs=2, name="l_rs")
                otr = [work.tile([128, SW], F32R, tag=f"otr{d}", bufs=1,
                                 name=f"otr{d}") for d in range(4)]
                bc = work.tile([128, SW], F32, tag="bc", bufs=2, name="bc")
                rsc = work.tile([128, SW], F32, tag="rsc", bufs=2, name="rsc")

                def part0(pl=deferred["pl"]):
                    nc.scalar.copy(l_rs, pl)
                    pb = psum("sim", name="pb")
                    mm(pb, ones_row, l_rs, start=True, stop=True)
                    nc.vector.reciprocal_approx_accurate(bc, pb, rsc)
                    for d in range(4):
                        nc.scalar.copy(otr[d], po[d])

                def part_cc(cc):
                    pf = psum("sim", name="pf")
                    for d in range(4):
                        mm(pf, wo[d][:, cc * 128:(cc + 1) * 128], otr[d],
                           start=(d == 0), stop=(d == 3))
                    for h in range(2):
                        hs = slice(h * 256, (h + 1) * 256)
                        cs = slice(s * SW + h * 256, s * SW + (h + 1) * 256)
                        fo = work.tile([128, 256], F32, tag="fo", bufs=4,
                                       name="fo")
                        nc.vector.tensor_mul(fo, pf[:, hs], bc[:, hs])
                        nc.vector.tensor_add(fo, fo, xqt[cc][:, cs])
                        nc.sync.dma_start(out[cc * 128:(cc + 1) * 128, cs], fo)

                return [part0, lambda: part_cc(0), lambda: part_cc(1)]

            for s in range(NSL):
                sl = slice(s * SW, (s + 1) * SW)
                po = [psum(f"po{d}", bufs=1, name=f"po{d}") for d in range(4)]
                lacc = work.tile([128, SW], F32R, tag="lacc", bufs=2,
                                 name="lacc")
                pts = []
                parts = deferred.pop("parts", None)

                def drain(j):
                    pt = pts[j]
                    for d in range(4):
                        mm(po[d], vt[j][:, d * 128:(d + 1) * 128], pt,
                           start=(j == 0), stop=(j == NJC - 1))

                for j in range(NJC):
                    ps = psum("sim", name="ps_s")
                    for d in range(4):
                        mm(ps, kt[d][:, j * 128:(j + 1) * 128], qT[d][:, sl],
                           start=(d == 0), stop=(d == 3))
                    pt = work.tile([128, SW], BF16, tag="pt", bufs=6, name="pt")
                    nc.scalar.activation(pt, ps, Exp, scale=SCALE)
                    pts.append(pt)
                    if j == 0:
                        nc.vector.tensor_copy(lacc, pt)
                    else:
                        nc.vector.tensor_add(lacc, lacc, pt)
                    if j >= KLATE:
                        drain(j - KLATE)
                    # previous slice's finalize, spread into this stream
                    if parts and j in (2, 4, 6):
                        parts.pop(0)()
                if s < NSL - 1:
                    for j in range(NJC - KLATE, NJC):
                        drain(j)
                    # denominator: one ones-matmul on the f32r running sum
                    pl = psum("pl", shape=(1, SW), bufs=1, name="pl")
                    mm(pl, ones_col, lacc, start=True, stop=True)
                    deferred["pl"] = pl
                    deferred["parts"] = finalize(s, po)
                else:
                    # last slice: interleave the tail drains with the
                    # denominator/projection chain to minimize exposure
                    drain(NJC - 3)
                    drain(NJC - 2)
                    pl = psum("pl", shape=(1, SW), bufs=1, name="pl")
                    mm(pl, ones_col, lacc, start=True, stop=True)
                    drain(NJC - 1)
                    deferred["pl"] = pl
                    for p in finalize(s, po):
                        p()

    nc.finalize()
    return nc


_NC_CACHE = None


def _get_nc():
    global _NC_CACHE
    if _NC_CACHE is None:
        _NC_CACHE = build_nc()
    return _NC_CACHE


def _round_f32r(a):
    """fp32 -> float32r rounding (round-half-even on the low 12 mantissa
    bits), matching the hardware's fp32_to_fp32r conversion."""
    bits = np.ascontiguousarray(a, dtype=np.float32).view(np.uint32)
    rem = bits & np.uint32(0xFFF)
    base = bits & np.uint32(0xFFFFF000)
    up = (rem > 0x800) | ((rem == 0x800) & (((bits >> np.uint32(12)) & np.uint32(1)) == 1))
    return (base + np.where(up, np.uint32(0x1000), np.uint32(0))).view(np.float32)


def prepare_in_maps(x, w_qkv, w_out, b_out):
    import ml_dtypes
    bf16 = ml_dtypes.bfloat16
    x = np.asarray(x, dtype=np.float32)
    w_qkv = np.asarray(w_qkv, dtype=np.float32)
    w_out = np.asarray(w_out, dtype=np.float32)
    b_out = np.asarray(b_out, dtype=np.float32)

    xr = x.reshape(B, C, N)
    wqkvT = np.ascontiguousarray(w_qkv.T).astype(bf16)   # [C, 1536]
    woutT = _round_f32r(np.ascontiguousarray(w_out.T))   # [512, C]
    bout = np.ascontiguousarray(b_out.reshape(2, 128, 1))

    in_maps = []
    for c in range(NCORES):
        b, h = divmod(c, 2)
        if h == 0:
            x_rot = xr[b]
        else:  # rotate so this core's query half sits in columns 0:HALF
            x_rot = np.concatenate([xr[b][:, HALF:], xr[b][:, :HALF]], axis=1)
        in_maps.append({
            "x_b": x_rot.astype(bf16),
            "xq_f": np.ascontiguousarray(x_rot[:, :HALF]),
            "wqkvT": wqkvT,
            "woutT": woutT,
            "bout": bout,
        })
    return in_maps


def postprocess(results):
    outs = [results[c]["out"] for c in range(NCORES)]
    full = np.stack([np.concatenate([outs[2 * b], outs[2 * b + 1]], axis=1)
                     for b in range(B)])               # [B, C, N]
    return full.reshape(B, C, 64, 64).astype(np.float32)


def kernel(x, w_qkv, w_out, b_out):
    in_maps = prepare_in_maps(x, w_qkv, w_out, b_out)
    res = run_bass_kernel_spmd(_get_nc(), in_maps, core_ids=list(range(NCORES)))
    return postprocess(res.results)
